# revision 44
# baseline (speedup 1.0000x reference)
"""Trainium2 Bass kernel for nn_CRPSSpectralLoss (v3).

Math (see reference.py):
  loss = crps_p + 0.1 * crps_f, each CRPS = mean|pred-tgt| - 0.5*(1-eps)*spread,
  spread = pairwise L1 over the M=16 ensemble; crps_f applies the same on
  |rfft2(x)| low-passed to the [kh<32, kw<16] corner.

Kernel strategy (8 cores, data-parallel over B; 1 sample per core):
  * max-trick: sum_{ordered pairs}|xi-xj| = 4*sum_{i<j}max(xi,xj) - 2(M-1)*sum xi
    and |a-b| = 2*max(a,b) - a - b, so ALL elementwise work is tensor_tensor
    max in fp16 (DVE 2x_1p mode) into packed scratch buffers.
  * Scratch reduced by ones-stationary matmuls (tensor engine) accumulating
    into PSUM; pointwise MAE reduced by ACT activation accum_out.
  * DMA: x split per channel, channel n+1 chained after channel n
    (full HBM bandwidth to the earliest-needed chunk); casts on DVE
    (fp32 tensor_copy runs 2x_2p) interleaved with the pair ops.
  * FFT corner via two DFT matmul stages; stage 2 uses tile_position column
    strips so |X| lands on 128 partitions; complex combine via PSUM
    accumulation; target rides as ensemble member 16; sum(x) = DC coeff.
  * Per-core output = raw partial-sum rows; host combines in float64.

Self-contained: hardcodes the problem shapes; imports numpy + concourse only.
"""

import numpy as np

B, M, C, H, W = 8, 16, 3, 128, 128
G = H * W
CUT_H, CUT_W = 32, 16
Gf = H * (W // 2 + 1)
LAMBDA_FREQ = 0.1
EPS = 0.05 / M

MT = M + 1            # ensemble members + target
USE_ACT_ACCUM = True  # pointwise-MAE reduction on ACT (False -> PE matmuls)

# res column offsets (partition 0)
OFF_PAIR = 0          # [0:512)    pointwise pair-max partials
OFF_MAE = 512         # [512:1024) pointwise max(x,t) partials
OFF_SPAIR = 1024      # [1024:1384) spectral pair-max partials (360)
OFF_STMAX = 1384      # [1384:1408) spectral max(|X|,|Xt|) partials (24)
OFF_S3F = 1408        # [1408:1792) sum|X| partials (384)
OFF_STF = 1792        # [1792:1816) sum|Xt| partials (24)
OFF_DC = 1816         # [1816:1867) DC per image: (c, m); m=16 -> target
RES_W = OFF_DC + 51


def dft_consts():
    h = np.arange(H)
    kh = np.arange(CUT_H)
    ang_h = 2 * np.pi * np.outer(h, kh) / H
    fh = np.concatenate([np.cos(ang_h), -np.sin(ang_h)], axis=1)
    w = np.arange(W)
    kw = np.arange(CUT_W)
    ang_w = 2 * np.pi * np.outer(w, kw) / W
    fw = np.concatenate(
        [np.cos(ang_w), -np.sin(ang_w), np.sin(ang_w)], axis=1
    )
    return fh.astype(np.float16), fw.astype(np.float16)


def build_nc():
    from contextlib import ExitStack

    from concourse import bacc, bass, mybir, tile
    from concourse.bass import _add_dep_helper

    f32 = mybir.dt.float32
    f16 = mybir.dt.float16
    MAX = mybir.AluOpType.max

    nc = bacc.Bacc("TRN2", target_bir_lowering=False, debug=False)

    x_dram = nc.declare_dram_parameter("x", [M, C, H, W], f32, isOutput=False)
    t_dram = nc.declare_dram_parameter("t", [C, H, W], f32, isOutput=False)
    fh_dram = nc.declare_dram_parameter("fh", [H, 2 * CUT_H], f16, isOutput=False)
    fw_dram = nc.declare_dram_parameter("fw", [W, 3 * CUT_W], f16, isOutput=False)
    res_dram = nc.declare_dram_parameter("res", [1, RES_W], f32, isOutput=True)

    PW_C = 15360        # per-channel packed pair scratch (sum_d (16-d)*128)
    SP_TOT = sum((MT - d) * 24 for d in range(1, MT))  # 3264

    with tile.TileContext(nc) as tc, ExitStack() as ctx:
        pool = ctx.enter_context(tc.tile_pool(name="main", bufs=1))
        xfp = ctx.enter_context(tc.tile_pool(name="xfp", bufs=2))
        ps1 = ctx.enter_context(
            tc.tile_pool(name="ps1", bufs=2, space=bass.MemorySpace.PSUM)
        )
        psf = ctx.enter_context(
            tc.tile_pool(name="psf", bufs=1, space=bass.MemorySpace.PSUM)
        )

        # ---- tiles ----
        x_h = pool.tile([128, M, C, W], f16)
        t_f = pool.tile([128, C, W], f32)
        t_h = pool.tile([128, C, W], f16)
        fh_sb = pool.tile([128, 2 * CUT_H], f16)
        fw_sb = pool.tile([128, 3 * CUT_W], f16)
        ones = pool.tile([128, 1], f16)
        ones_f = pool.tile([128, 1], f32)
        y_h = pool.tile([128, M * C + C, 2, CUT_H], f16)
        xm = pool.tile([128, C, MT, 8], f16)
        pw_all = pool.tile([128, C, PW_C], f16)
        mae_all = pool.tile([128, C, M * W], f16)
        sp_all = pool.tile([128, SP_TOT], f16)
        sq_re = pool.tile([128, C * MT * 8], f32)
        sq_im = pool.tile([128, C * MT * 8], f32)
        s2 = pool.tile([128, C * MT * 8], f32)
        mae_acc = pool.tile([128, C], f32)
        dc_all = pool.tile([1, C, MT], f32)
        fin = pool.tile([1, RES_W], f32)

        ps_pair = psf.tile([1, 512], f32, tag="ps_pair")
        ps_mae = psf.tile([1, 512], f32, tag="ps_mae")
        ps_m1 = psf.tile([1, 512], f32, tag="ps_m1")
        ps_m2 = psf.tile([1, 512], f32, tag="ps_m2")
        psum_re_f = psf.tile([128, 512], f32, tag="psum_re")
        psum_im_f = psf.tile([128, 512], f32, tag="psum_im")
        psum_re = psum_re_f[:, 0 : C * MT * 8].rearrange(
            "p (c m k) -> p c m k", c=C, m=MT)
        psum_im = psum_im_f[:, 0 : C * MT * 8].rearrange(
            "p (c m k) -> p c m k", c=C, m=MT)

        # ---- DMA: per-channel, halves on two queues, channels chained so
        # the earliest-needed channel gets the full HBM bandwidth ----
        xr = x_dram.ap().rearrange("m c h w -> h m c w")
        x_fs = []
        prev = None
        for c in range(C):
            xf = xfp.tile([128, M, W], f32, name=f"xf{c}", tag="xf")
            eng = (nc.sync, nc.scalar)[c % 2]
            d1 = eng.dma_start(out=xf[:, 0 : M // 2, :], in_=xr[:, 0 : M // 2, c, :])
            d2 = eng.dma_start(out=xf[:, M // 2 :, :], in_=xr[:, M // 2 :, c, :])
            if prev is not None:
                for dd in (d1, d2):
                    for pp in prev:
                        _add_dep_helper(dd.ins, pp.ins, sync=True,
                                        reason="serialize x channels for bw")
            prev = (d1, d2)
            x_fs.append(xf)
        nc.scalar.dma_start(out=t_f[:], in_=t_dram.ap().rearrange("c h w -> h c w"))
        nc.sync.dma_start(out=fh_sb[:], in_=fh_dram.ap())
        nc.sync.dma_start(out=fw_sb[:], in_=fw_dram.ap())
        nc.gpsimd.memset(ones[:], 1.0)
        nc.gpsimd.memset(ones_f[:], 1.0)
        nc.vector.memset(psum_re_f[:], 0.0)
        nc.vector.memset(psum_im_f[:], 0.0)

        # ---- DVE: cast + pair maxes, channel-interleaved ----
        pair_off = [0]
        for d in range(1, M):
            pair_off.append(pair_off[-1] + (M - d) * W)
        for c in range(C):
            nc.vector.tensor_copy(out=x_h[:, :, c, :], in_=x_fs[c][:])
            for d in range(1, M):
                n = (M - d) * W
                nc.vector.tensor_tensor(
                    out=pw_all[:, c, pair_off[d - 1] : pair_off[d - 1] + n]
                        .rearrange("p (m w) -> p m w", m=M - d),
                    in0=x_h[:, : M - d, c, :], in1=x_h[:, d:, c, :], op=MAX)
        nc.vector.tensor_copy(out=t_h[:], in_=t_f[:])
        for c in range(C):
            nc.vector.tensor_tensor(
                out=mae_all[:, c, :].rearrange("p (m w) -> p m w", m=M),
                in0=x_h[:, :, c, :],
                in1=t_h[:, c, :].unsqueeze(1).broadcast_to((128, M, W)),
                op=MAX)

        # ---- FFT stage 1 (PE; c-ordered so it starts after cast c0) ----
        for g in range(6):
            y_ps = ps1.tile([128, 512], f32, tag="y_ps")
            for k in range(8):
                s = g * 8 + k
                c, m = s // M, s % M
                nc.tensor.matmul(y_ps[:, k * 64 : (k + 1) * 64],
                                 x_h[:, m, c, :], fh_sb[:],
                                 start=True, stop=True)
            nc.scalar.copy(out=y_h[:, g * 8 : (g + 1) * 8, :, :], in_=y_ps[:])
        y_pst = ps1.tile([128, 512], f32, tag="y_ps")
        for c in range(C):
            nc.tensor.matmul(y_pst[:, c * 64 : (c + 1) * 64],
                             t_h[:, c, :], fh_sb[:], start=True, stop=True)
        nc.scalar.copy(out=y_h[:, M * C : M * C + C, :, :], in_=y_pst[:, 0:192])

        # ---- FFT stage 2 (PE, column strips) ----
        fwre, fwim, fwimn = fw_sb[:, 0:16], fw_sb[:, 16:32], fw_sb[:, 32:48]
        for q in range(4):
            tp = (0, 32 * q)
            lo, hi = 32 * q, 32 * q + 16
            khs = slice(q * 8, (q + 1) * 8)
            for c in range(C):
                yre = y_h[:, c * M : (c + 1) * M, 0, khs]
                yim = y_h[:, c * M : (c + 1) * M, 1, khs]
                o_re = psum_re[lo:hi, c, 0:M, :].rearrange("p m k -> p (m k)")
                o_im = psum_im[lo:hi, c, 0:M, :].rearrange("p m k -> p (m k)")
                nc.tensor.matmul(o_re, fwre, yre, start=True, stop=False,
                                 tile_position=tp)
                nc.tensor.matmul(o_re, fwimn, yim, start=False, stop=True,
                                 tile_position=tp)
                nc.tensor.matmul(o_im, fwim, yre, start=True, stop=False,
                                 tile_position=tp)
                nc.tensor.matmul(o_im, fwre, yim, start=False, stop=True,
                                 tile_position=tp)
            for c in range(C):
                ytre = y_h[:, M * C + c, 0, khs]
                ytim = y_h[:, M * C + c, 1, khs]
                nc.tensor.matmul(psum_re[lo:hi, c, M, :], fwre, ytre,
                                 start=True, stop=False, tile_position=tp)
                nc.tensor.matmul(psum_re[lo:hi, c, M, :], fwimn, ytim,
                                 start=False, stop=True, tile_position=tp)
                nc.tensor.matmul(psum_im[lo:hi, c, M, :], fwim, ytre,
                                 start=True, stop=False, tile_position=tp)
                nc.tensor.matmul(psum_im[lo:hi, c, M, :], fwre, ytim,
                                 start=False, stop=True, tile_position=tp)

        # DC per image (strip q=0, kw=0, khsub=0 -> partition 0)
        nc.scalar.copy(out=dc_all[:], in_=psum_re[0:1, :, :, 0])

        # |X| in one square/add/sqrt chain over (128, 408)
        nc.scalar.square(out=sq_re[:], in_=psum_re_f[:, 0 : C * MT * 8])
        nc.scalar.square(out=sq_im[:], in_=psum_im_f[:, 0 : C * MT * 8])
        nc.gpsimd.tensor_add(s2[:], sq_re[:], sq_im[:])
        nc.scalar.sqrt(out=xm[:].rearrange("p c m k -> p (c m k)"), in_=s2[:])

        # ---- spectral pair maxes (DVE, 17 members: [x-pairs | t-pair]) ----
        sp_off = [0]
        for d in range(1, MT):
            sp_off.append(sp_off[-1] + (MT - d) * 24)
        for d in range(1, MT):
            nm = MT - d
            nc.vector.tensor_tensor(
                out=sp_all[:, sp_off[d - 1] : sp_off[d]].rearrange(
                    "p (c n) -> p c n", c=C),
                in0=xm[:, :, :nm, :].rearrange("p c m k -> p c (m k)"),
                in1=xm[:, :, d:, :].rearrange("p c m k -> p c (m k)"),
                op=MAX)

        # ---- pointwise reductions ----
        pair_chunks = []
        for c in range(C):
            for off in range(0, PW_C, 512):
                pair_chunks.append((c, off, min(512, PW_C - off)))
        for i, (c, off, w) in enumerate(pair_chunks):
            nc.tensor.matmul(ps_pair[:, :w], ones[:], pw_all[:, c, off : off + w],
                             start=(i == 0), stop=(i == len(pair_chunks) - 1))
        if USE_ACT_ACCUM:
            from concourse import mybir as _mb
            for c in range(C):
                nc.scalar.activation(
                    out=mae_all[:, c, :], in_=mae_all[:, c, :],
                    func=_mb.ActivationFunctionType.Identity,
                    accum_out=mae_acc[:, c : c + 1])
            nc.tensor.matmul(ps_mae[:, 0:C], ones_f[:], mae_acc[:],
                             start=True, stop=True)
        else:
            mchunks = [(c, off) for c in range(C)
                       for off in range(0, M * W, 512)]
            for i, (c, off) in enumerate(mchunks):
                nc.tensor.matmul(ps_mae[:, 0:512], ones[:],
                                 mae_all[:, c, off : off + 512],
                                 start=(i == 0), stop=(i == len(mchunks) - 1))

        # ---- spectral reductions (PE; two sequential groups per bank) ----
        def spd_view(d):
            return sp_all[:, sp_off[d - 1] : sp_off[d]].rearrange(
                "p (c m k) -> p c m k", c=C, m=MT - d)

        for d in range(1, M):
            nc.tensor.matmul(
                ps_m1[:, : (M - d) * 24], ones[:],
                spd_view(d)[:, :, : M - d, :].rearrange("p c m k -> p c (m k)"),
                start=(d == 1), stop=(d == 15))
        for d in range(1, MT):
            nc.tensor.matmul(ps_m1[:, 360:384], ones[:],
                             spd_view(d)[:, :, M - d, :],
                             start=(d == 1), stop=(d == MT - 1))
        nc.tensor.matmul(ps_m2[:, 0:384], ones[:],
                         xm[:, :, 0:M, :].rearrange("p c m k -> p c (m k)"),
                         start=True, stop=True)
        nc.tensor.matmul(ps_m2[:, 384:408], ones[:], xm[:, :, M, :],
                         start=True, stop=True)

        # ---- pack + output ----
        nc.scalar.copy(out=fin[:, OFF_PAIR : OFF_PAIR + 512], in_=ps_pair[:])
        if USE_ACT_ACCUM:
            nc.scalar.copy(out=fin[:, OFF_MAE : OFF_MAE + C], in_=ps_mae[:, 0:C])
            nc.gpsimd.memset(fin[:, OFF_MAE + C : OFF_MAE + 512], 0.0)
        else:
            nc.scalar.copy(out=fin[:, OFF_MAE : OFF_MAE + 512], in_=ps_mae[:])
        nc.scalar.copy(out=fin[:, OFF_SPAIR : OFF_SPAIR + 384], in_=ps_m1[:, 0:384])
        nc.scalar.copy(out=fin[:, OFF_S3F : OFF_S3F + 408], in_=ps_m2[:, 0:408])
        nc.scalar.copy(out=fin[:, OFF_DC : OFF_DC + C * MT],
                       in_=dc_all[:].rearrange("p c m -> p (c m)"))
        nc.sync.dma_start(out=res_dram.ap(), in_=fin[:])

    nc.compile()
    return nc


_NC_CACHE = None


def _get_nc():
    global _NC_CACHE
    if _NC_CACHE is None:
        _NC_CACHE = build_nc()
    return _NC_CACHE


def sums_from_res(res):
    r = np.asarray(res, dtype=np.float64).reshape(-1)
    dc = r[OFF_DC : OFF_DC + C * MT].reshape(C, MT)
    return dict(
        S_pairmax=r[OFF_PAIR : OFF_PAIR + 512].sum(),
        S_maxt=r[OFF_MAE : OFF_MAE + 512].sum(),
        S3=dc[:, 0:M].sum(),
        S_t=dc[:, M].sum(),
        Sf_pairmax=r[OFF_SPAIR : OFF_SPAIR + 360].sum(),
        Sf_maxt=r[OFF_STMAX : OFF_STMAX + 24].sum(),
        S3f=r[OFF_S3F : OFF_S3F + 384].sum(),
        S_tf=r[OFF_STF : OFF_STF + 24].sum(),
    )


def combine_sums(sums_list):
    tot = {k: sum(s[k] for s in sums_list) for k in sums_list[0]}
    P_pt = C * G
    mae_pt = 2 * tot['S_maxt'] - tot['S3'] - M * tot['S_t']
    spread_pt = 4 * tot['S_pairmax'] - 2 * (M - 1) * tot['S3']
    term1_p = mae_pt / (B * M * P_pt)
    term2_p = spread_pt / ((M - 1) * B * M * P_pt) * (1 - EPS)
    crps_p = term1_p - 0.5 * term2_p

    P_f = C * Gf
    mae_f = 2 * tot['Sf_maxt'] - tot['S3f'] - M * tot['S_tf']
    spread_f = 4 * tot['Sf_pairmax'] - 2 * (M - 1) * tot['S3f']
    term1_f = mae_f / (B * M * P_f)
    term2_f = spread_f / ((M - 1) * B * M * P_f) * (1 - EPS)
    crps_f = term1_f - 0.5 * term2_f

    return np.float32(crps_p + LAMBDA_FREQ * crps_f)


def combine_results(res_list):
    return combine_sums([sums_from_res(res) for res in res_list])


def make_in_maps(target, output):
    fh, fw = dft_consts()
    target = np.ascontiguousarray(np.asarray(target, dtype=np.float32))
    output = np.ascontiguousarray(np.asarray(output, dtype=np.float32))
    return [
        {"x": output[b], "t": target[b], "fh": fh, "fw": fw}
        for b in range(B)
    ]


def kernel(target, output):
    from concourse.bass_utils import run_bass_kernel_spmd

    nc = _get_nc()
    in_maps = make_in_maps(target, output)
    results = run_bass_kernel_spmd(nc, in_maps, list(range(B))).results
    return combine_results([results[b]["res"] for b in range(B)])
